# revision 27
# baseline (speedup 1.0000x reference)
"""AttnCutLoss Trainium2 kernel (v3).

Reference math (B=4096 rows, S=4096 positions, f1 metric, tau=0.95):
    tp    = cumsum(labels, axis=1)
    r     = 2*tp / (k + total)          [exact algebraic form of the f1 weight,
                                         incl. the tp==0 / total==0 guards]
    q     = exp(r/tau); norm = sum_j q; w = 1/norm
    loss  = -sum(log(output)*w)/B = -(1/B) * sum_rows [ (sum_j log(output)) / norm ]

Approximations (tolerance is rel 2e-2; these land ~1e-3 combined):
  * labels are pre-folded Fx on host (F=16): lab16[i] = sum of a 16-block.
    cumsum gives tp at k=16,32,...,4096 EXACTLY; norm ~= 16*sum_i f(16i).
    f = exp(2.105*tp/(T+k)) spans only [1, ~1.43] so the right-endpoint
    Riemann bias is ~+6e-4 relative. 16x fewer scan/recip/mult/exp elems.
  * output is sent as fp16 scaled by 32 (no fp16 subnormals after the first
    pairwise product; ln rel err ~2^-12).
  * log-sum pass is quartered by two pairwise-product folds:
    sum ln(x) = sum ln((a*b)*(c*d)); fold1 fp16 (DVE 2x TT mode),
    fold2 f32 (products up to 32^4 overflow fp16). Host subtracts the exact
    S*ln(32) scale correction per row.

Per-core engine split (512 rows/core, 4 groups of 128 partitions):
  DVE : scan; reciprocal_approx_fast; r=tp*inv; fold1; fold2 (or half)
  ACT : d = k+T via Identity(bias=T); Ln(fold2) accum; Exp(r*2/tau) accum
        (single act-table set 6 serves Ln+Exp: no in-loop table reloads)
  POOL: optional half of fold2 (plain TensorTensor only; TensorScalarPtr on
        Pool costs ~6.5us/op on HW - never use it there)
  DMA : output fp16 4MB/core + labels 0.25MB/core, optionally split across
        the SP and ACT HWDGE rings
Host: loss = -(sum over rows (logsum_row - S*ln 32)/(F*normacc_row))/B.
"""

import numpy as np
import ml_dtypes

B = 4096
S = 4096
TAU = 0.95
NCORES = 8
RPC = B // NCORES          # rows per core = 512
G = RPC // 128             # row groups per core = 4
F = 16                     # host fold factor for labels
SF = S // F                # folded row length = 256
OSCALE = 64.0              # host scale on output before fp8/fp16 cast
USE_FP8 = True             # send output as fp8 e4m3 (else fp16)
LNCORR = S * float(np.log(OSCALE))  # per-row logsum correction

_PROGRAM_CACHE = {}


def _build_program(repeats: int = 1, d_eng: str = "act", r_eng: str = "dve",
                   dma_only: bool = False, static_dma: bool = False,
                   fold2: bool = True, fold2_eng: str = "pool",
                   split_rings: bool = True, dma_pack: int = 1,
                   fp8: bool = USE_FP8):
    import concourse.bass as bass
    import concourse.tile as tile
    import concourse.mybir as mybir
    from concourse import bacc
    from contextlib import ExitStack
    import contextlib

    dt = mybir.dt
    alu = mybir.AluOpType
    act = mybir.ActivationFunctionType

    nc = bacc.Bacc("TRN2")
    out_dt = dt.float8e4 if fp8 else dt.float16
    # row-major [RPC, S]: group g = rows [g*128,(g+1)*128) -> contiguous block
    outh = nc.dram_tensor("outh", [RPC, S], out_dt, kind="ExternalInput")
    lab8 = nc.dram_tensor("lab8", [128, G * SF], dt.float16, kind="ExternalInput")
    kt = nc.dram_tensor("kt", [128, SF], dt.float32, kind="ExternalInput")
    norms = nc.dram_tensor("norms", [128, G], dt.float32, kind="ExternalOutput")
    logsums = nc.dram_tensor("logsums", [128, G], dt.float32, kind="ExternalOutput")

    HF = S // 2            # fold1 width
    QF = S // 4            # fold2 width

    with ExitStack() as ctx:
        tc = ctx.enter_context(tile.TileContext(nc))
        consts = ctx.enter_context(tc.tile_pool(name="consts", bufs=1))
        labp = ctx.enter_context(tc.tile_pool(name="labp", bufs=1))
        outp = ctx.enter_context(tc.tile_pool(name="outp", bufs=4 if static_dma else 3))
        tpp = ctx.enter_context(tc.tile_pool(name="tpp", bufs=2))
        dp = ctx.enter_context(tc.tile_pool(name="dp", bufs=2))
        invp = ctx.enter_context(tc.tile_pool(name="invp", bufs=2))
        rp = ctx.enter_context(tc.tile_pool(name="rp", bufs=4))
        foldp = ctx.enter_context(tc.tile_pool(name="foldp", bufs=2))
        fold2p = ctx.enter_context(tc.tile_pool(name="fold2p", bufs=4))
        dump = ctx.enter_context(tc.tile_pool(name="dump", bufs=1))
        accp = ctx.enter_context(tc.tile_pool(name="accp", bufs=1))

        # Pre-load ACT table set 6 (natural_log_exp_and_others): serves BOTH
        # Ln and Exp, so the act-table-load pass inserts no in-loop reloads.
        _li = mybir.InstLoadActFuncSet(
            name=nc.get_next_instruction_name(), ins=[], outs=[])
        _li.act_func_set_id = 6
        nc.scalar.add_instruction(_li)

        kt_sb = consts.tile([128, SF], dt.float32)
        nc.sync.dma_start(kt_sb[:, :], kt[:, :])

        naccs_sb = accp.tile([128, G], dt.float32)
        logsums_sb = accp.tile([128, G], dt.float32)
        qdump = dump.tile([128, SF], dt.bfloat16)
        ldump = dump.tile([128, QF if fold2 else HF], dt.bfloat16)

        def out_dma(g, tile_t):
            eng = nc.scalar if (split_rings and g % 2 == 1) else nc.sync
            eng.dma_start(tile_t[:, :], outh[g * 128:(g + 1) * 128, :])

        static_outs = []
        if static_dma:
            lab_t = labp.tile([128, G * SF], dt.float16, tag="lab")
            nc.sync.dma_start(lab_t[:, :], lab8[:, :])
            for g in range(G):
                sout = outp.tile([128, S], out_dt, tag="outv")
                out_dma(g, sout)
                static_outs.append(sout)

        loop_cm = tc.For_i(0, repeats, 1) if repeats > 1 else contextlib.nullcontext()
        with loop_cm:
            if not static_dma:
                lab_t = labp.tile([128, G * SF], dt.float16, tag="lab")
                nc.sync.dma_start(lab_t[:, :], lab8[:, :])
            if dma_only:
                assert dma_pack in (1, 2, 4)
                npk = G // dma_pack
                for i in range(npk):
                    out_t = outp.tile([128, S * dma_pack], out_dt, tag="outv")
                    eng = nc.scalar if (split_rings and i % 2 == 1) else nc.sync
                    src = outh[i * 128 * dma_pack:(i + 1) * 128 * dma_pack, :]
                    if dma_pack > 1:
                        src = src.rearrange("(p k) s -> p (k s)", k=dma_pack)
                    eng.dma_start(out_t[:, :], src)
            fold_ts = []
            r_ts = []
            for g in range(G if not dma_only else 0):
                if static_dma:
                    out_t = static_outs[g]
                else:
                    out_t = outp.tile([128, S], out_dt, tag="outv")
                    out_dma(g, out_t)

                # tp = cumsum(lab) along free dim; exact integers
                tp_t = tpp.tile([128, SF], dt.float32, tag="tp")
                nc.vector.tensor_tensor_scan(
                    tp_t[:, :], lab_t[:, g * SF:(g + 1) * SF],
                    lab_t[:, g * SF:(g + 1) * SF], 0.0, alu.add, alu.bypass
                )

                # d = k + T  (T = tp[:, -1], per-partition scalar)
                d_t = dp.tile([128, SF], dt.float32, tag="d")
                if d_eng == "act":
                    nc.scalar.activation(
                        d_t[:, :], kt_sb[:, :], act.Identity,
                        bias=tp_t[:, SF - 1:SF], scale=1.0)
                else:
                    deng = nc.gpsimd if d_eng == "pool" else nc.vector
                    deng.tensor_scalar_add(d_t[:, :], kt_sb[:, :],
                                           tp_t[:, SF - 1:SF])

                # inv = 1/d on DVE (approx, ~51 ULP)
                inv_t = invp.tile([128, SF], dt.float32, tag="inv")
                nc.vector.reciprocal_approx_fast(out=inv_t[:, :], in_=d_t[:, :])

                # fold1: prod = out[:, :HF] * out[:, HF:]  (fp16 2x TT mode)
                fold_t = foldp.tile([128, HF], dt.float16, tag="fold")
                nc.vector.tensor_tensor(
                    fold_t[:, :], out_t[:, :HF], out_t[:, HF:], alu.mult
                )

                if fold2:
                    f2_t = fold2p.tile([128, QF], dt.float32, tag="fold2")
                    f2eng = nc.gpsimd if fold2_eng == "pool" else nc.vector
                    f2eng.tensor_tensor(
                        f2_t[:, :], fold_t[:, :QF], fold_t[:, QF:], alu.mult)
                    fold_ts.append(f2_t)
                else:
                    fold_ts.append(fold_t)

                # r = tp * inv
                r_t = rp.tile([128, SF], dt.float32, tag="r")
                reng = nc.gpsimd if r_eng == "pool" else nc.vector
                reng.tensor_tensor(
                    r_t[:, :], tp_t[:, :], inv_t[:, :], alu.mult
                )
                r_ts.append(r_t)

            # ACT phase: batch all Ln then all Exp
            for g in range(G if not dma_only else 0):
                nc.scalar.activation(
                    ldump[:, :], fold_ts[g][:, :], act.Ln,
                    accum_out=logsums_sb[:, g:g + 1],
                )
            for g in range(G if not dma_only else 0):
                nc.scalar.activation(
                    qdump[:, :], r_ts[g][:, :], act.Exp,
                    scale=2.0 / TAU,
                    accum_out=naccs_sb[:, g:g + 1],
                )

        if not dma_only:
            nc.sync.dma_start(norms[:, :], naccs_sb[:, :])
            nc.sync.dma_start(logsums[:, :], logsums_sb[:, :])

    nc.finalize()
    return nc


def _make_consts():
    k = (np.arange(1, SF + 1, dtype=np.float32) * F)  # 16, 32, ..., 4096
    kt = np.ascontiguousarray(np.broadcast_to(k, (128, SF))).astype(np.float32)
    return kt


def _prep_inputs(output, labels):
    """Host-side shard + dtype/layout prep. Returns per-core in_maps."""
    output = np.asarray(output)
    labels = np.asarray(labels)
    assert output.shape == (B, S, 1) and labels.shape == (B, S)

    out_np_dt = ml_dtypes.float8_e4m3 if USE_FP8 else np.float16
    outh_full = (output.reshape(B, S).astype(np.float32, copy=False) * OSCALE
                 ).astype(out_np_dt)
    # fold labels Fx: integer counts 0..F, exact in fp16
    lab8_full = labels.reshape(B, SF, F).sum(axis=2, dtype=np.float32
                                             ).astype(np.float16)

    kt = _make_consts()
    in_maps = []
    for c in range(NCORES):
        sl = slice(c * RPC, (c + 1) * RPC)
        # outh row-major [RPC, S] (group g = row block, contiguous 1MB DMA);
        # lab8 [128 partitions, G*SF]: col-block g = rows g*128..g*128+127
        lab8_c = np.ascontiguousarray(
            lab8_full[sl].reshape(G, 128, SF).transpose(1, 0, 2).reshape(128, G * SF))
        in_maps.append({
            "outh": np.ascontiguousarray(outh_full[sl]),
            "lab8": lab8_c,
            "kt": kt,
        })
    return in_maps


def _postprocess(res):
    total = 0.0
    for c in range(NCORES):
        naccs = np.asarray(res.results[c]["norms"], dtype=np.float64)
        logs = np.asarray(res.results[c]["logsums"], dtype=np.float64)
        total += float(np.sum((logs - LNCORR) / (F * naccs)))
    return np.float32(-total / B)


def _run(output, labels, trace=False):
    from concourse.bass_utils import run_bass_kernel_spmd

    if "prog" not in _PROGRAM_CACHE:
        _PROGRAM_CACHE["prog"] = _build_program()
    nc = _PROGRAM_CACHE["prog"]

    in_maps = _prep_inputs(output, labels)
    res = run_bass_kernel_spmd(nc, in_maps, core_ids=list(range(NCORES)),
                               trace=trace)
    return _postprocess(res), res


def kernel(output, labels):
    loss, _ = _run(output, labels, trace=False)
    return loss


# revision 34
# speedup vs baseline: 1.1599x; 1.1599x over previous
"""AttnCutLoss Trainium2 kernel (v3).

Reference math (B=4096 rows, S=4096 positions, f1 metric, tau=0.95):
    tp    = cumsum(labels, axis=1)
    r     = 2*tp / (k + total)          [exact algebraic form of the f1 weight,
                                         incl. the tp==0 / total==0 guards]
    q     = exp(r/tau); norm = sum_j q; w = 1/norm
    loss  = -sum(log(output)*w)/B = -(1/B) * sum_rows [ (sum_j log(output)) / norm ]

Approximations (tolerance is rel 2e-2; these land ~1e-3 combined):
  * labels are pre-folded Fx on host (F=16): lab16[i] = sum of a 16-block.
    cumsum gives tp at k=16,32,...,4096 EXACTLY; norm ~= 16*sum_i f(16i).
    f = exp(2.105*tp/(T+k)) spans only [1, ~1.43] so the right-endpoint
    Riemann bias is ~+6e-4 relative. 16x fewer scan/recip/mult/exp elems.
  * output is sent as fp16 scaled by 32 (no fp16 subnormals after the first
    pairwise product; ln rel err ~2^-12).
  * log-sum pass is quartered by two pairwise-product folds:
    sum ln(x) = sum ln((a*b)*(c*d)); fold1 fp16 (DVE 2x TT mode),
    fold2 f32 (products up to 32^4 overflow fp16). Host subtracts the exact
    S*ln(32) scale correction per row.

Per-core engine split (512 rows/core, 4 groups of 128 partitions):
  DVE : scan; reciprocal_approx_fast; r=tp*inv; fold1; fold2 (or half)
  ACT : d = k+T via Identity(bias=T); Ln(fold2) accum; Exp(r*2/tau) accum
        (single act-table set 6 serves Ln+Exp: no in-loop table reloads)
  POOL: optional half of fold2 (plain TensorTensor only; TensorScalarPtr on
        Pool costs ~6.5us/op on HW - never use it there)
  DMA : output fp16 4MB/core + labels 0.25MB/core, optionally split across
        the SP and ACT HWDGE rings
Host: loss = -(sum over rows (logsum_row - S*ln 32)/(F*normacc_row))/B.
"""

import numpy as np
import ml_dtypes

B = 4096
S = 4096
TAU = 0.95
NCORES = 8
RPC = B // NCORES          # rows per core = 512
G = RPC // 128             # row groups per core = 4
F = 16                     # host fold factor for labels
SF = S // F                # folded row length = 256
OSCALE = 64.0              # host scale on output before fp8/fp16 cast
USE_FP8 = True             # send output as fp8 e4m3 (else fp16)
LNCORR = S * float(np.log(OSCALE))  # per-row logsum correction

_PROGRAM_CACHE = {}


def _build_program(repeats: int = 1, d_eng: str = "act", r_eng: str = "dve",
                   dma_only: bool = False, static_dma: bool = False,
                   fold2: bool = False, fold2_eng: str = "dve",
                   split_rings: bool = True, dma_pack: int = 2,
                   fp8: bool = USE_FP8, host_inv: bool = False,
                   outp_bufs: int = 3, dma_first: bool = False,
                   fold1: bool = True):
    import concourse.bass as bass
    import concourse.tile as tile
    import concourse.mybir as mybir
    from concourse import bacc
    from contextlib import ExitStack
    import contextlib

    dt = mybir.dt
    alu = mybir.AluOpType
    act = mybir.ActivationFunctionType

    nc = bacc.Bacc("TRN2")
    out_dt = dt.float8e4 if fp8 else dt.float16
    # row-major [RPC, S]: group g = rows [g*128,(g+1)*128) -> contiguous block
    outh = nc.dram_tensor("outh", [RPC, S], out_dt, kind="ExternalInput")
    lab8 = nc.dram_tensor("lab8", [128, G * SF], dt.float16, kind="ExternalInput")
    invt = nc.dram_tensor("invt", [128, G * SF], dt.float16, kind="ExternalInput")
    kt = nc.dram_tensor("kt", [128, SF], dt.float32, kind="ExternalInput")
    norms = nc.dram_tensor("norms", [128, G], dt.float32, kind="ExternalOutput")
    logsums = nc.dram_tensor("logsums", [128, G], dt.float32, kind="ExternalOutput")

    HF = S // 2            # fold1 width
    QF = S // 4            # fold2 width

    with ExitStack() as ctx:
        tc = ctx.enter_context(tile.TileContext(nc))
        consts = ctx.enter_context(tc.tile_pool(name="consts", bufs=1))
        labp = ctx.enter_context(tc.tile_pool(name="labp", bufs=1))
        outp = ctx.enter_context(tc.tile_pool(
            name="outp", bufs=4 if static_dma else outp_bufs))
        tpp = ctx.enter_context(tc.tile_pool(name="tpp", bufs=2))
        dp = ctx.enter_context(tc.tile_pool(name="dp", bufs=2))
        invp = ctx.enter_context(tc.tile_pool(name="invp", bufs=2))
        rp = ctx.enter_context(tc.tile_pool(name="rp", bufs=4))
        foldp = ctx.enter_context(tc.tile_pool(name="foldp", bufs=2))
        fold2p = ctx.enter_context(tc.tile_pool(name="fold2p", bufs=4))
        dump = ctx.enter_context(tc.tile_pool(name="dump", bufs=1))
        accp = ctx.enter_context(tc.tile_pool(name="accp", bufs=1))

        # Pre-load ACT table set 6 (natural_log_exp_and_others): serves BOTH
        # Ln and Exp, so the act-table-load pass inserts no in-loop reloads.
        _li = mybir.InstLoadActFuncSet(
            name=nc.get_next_instruction_name(), ins=[], outs=[])
        _li.act_func_set_id = 6
        nc.scalar.add_instruction(_li)

        kt_sb = consts.tile([128, SF], dt.float32)
        nc.sync.dma_start(kt_sb[:, :], kt[:, :])

        naccs_sb = accp.tile([128, G], dt.float32)
        logsums_sb = accp.tile([128, G], dt.float32)
        qdump = dump.tile([128, SF], dt.bfloat16)
        lnw = (QF if fold2 else HF) if fold1 else S
        ldump = dump.tile([128, lnw], dt.bfloat16)

        def out_dma(g, tile_t):
            eng = nc.scalar if (split_rings and g % 2 == 1) else nc.sync
            eng.dma_start(tile_t[:, :], outh[g * 128:(g + 1) * 128, :])

        static_outs = []
        if static_dma:
            lab_t = labp.tile([128, G * SF], dt.float16, tag="lab")
            nc.sync.dma_start(lab_t[:, :], lab8[:, :])
            for g in range(G):
                sout = outp.tile([128, S], out_dt, tag="outv")
                out_dma(g, sout)
                static_outs.append(sout)

        loop_cm = tc.For_i(0, repeats, 1) if repeats > 1 else contextlib.nullcontext()
        with loop_cm:
            if not static_dma:
                lab_t = labp.tile([128, G * SF], dt.float16, tag="lab")
                nc.sync.dma_start(lab_t[:, :], lab8[:, :])
            if host_inv:
                inv_all = labp.tile([128, G * SF], dt.float16, tag="invh")
                nc.sync.dma_start(inv_all[:, :], invt[:, :])
            if dma_only:
                assert dma_pack in (1, 2, 4)
                npk = G // dma_pack
                for i in range(npk):
                    out_t = outp.tile([128, S * dma_pack], out_dt, tag="outv")
                    eng = nc.scalar if (split_rings and i % 2 == 1) else nc.sync
                    src = outh[i * 128 * dma_pack:(i + 1) * 128 * dma_pack, :]
                    if dma_pack > 1:
                        src = src.rearrange("(k p) s -> p k s", k=dma_pack)
                    eng.dma_start(out_t[:, :], src)
            fold_ts = []
            r_ts = []
            pre_outs = []
            if dma_first and not static_dma and not dma_only:
                for g in range(G):
                    out_t = outp.tile([128, S], out_dt, tag="outv")
                    out_dma(g, out_t)
                    pre_outs.append(out_t)
            if dma_pack > 1 and not static_dma and not dma_only:
                # packed DMAs: tile cols [j*S:(j+1)*S] = group i*pack+j
                for i in range(G // dma_pack):
                    pt = outp.tile([128, S * dma_pack], out_dt, tag="outv")
                    eng = nc.scalar if (split_rings and i % 2 == 1) else nc.sync
                    src = outh[i * 128 * dma_pack:(i + 1) * 128 * dma_pack, :]
                    src = src.rearrange("(k p) s -> p k s", k=dma_pack)
                    eng.dma_start(pt[:, :], src)
                    pre_outs.append(pt)
            for g in range(G if not dma_only else 0):
                if static_dma:
                    out_t = static_outs[g]
                elif dma_first:
                    out_t = pre_outs[g]
                elif dma_pack > 1:
                    out_t = pre_outs[g // dma_pack][
                        :, (g % dma_pack) * S:(g % dma_pack + 1) * S]
                else:
                    out_t = outp.tile([128, S], out_dt, tag="outv")
                    out_dma(g, out_t)

                # tp = cumsum(lab) along free dim; exact integers
                tp_t = tpp.tile([128, SF], dt.float32, tag="tp")
                nc.vector.tensor_tensor_scan(
                    tp_t[:, :], lab_t[:, g * SF:(g + 1) * SF],
                    lab_t[:, g * SF:(g + 1) * SF], 0.0, alu.add, alu.bypass
                )

                if host_inv:
                    inv_ap = inv_all[:, g * SF:(g + 1) * SF]
                else:
                    # d = k + T  (T = tp[:, -1], per-partition scalar)
                    d_t = dp.tile([128, SF], dt.float32, tag="d")
                    if d_eng == "act":
                        nc.scalar.activation(
                            d_t[:, :], kt_sb[:, :], act.Identity,
                            bias=tp_t[:, SF - 1:SF], scale=1.0)
                    else:
                        deng = nc.gpsimd if d_eng == "pool" else nc.vector
                        deng.tensor_scalar_add(d_t[:, :], kt_sb[:, :],
                                               tp_t[:, SF - 1:SF])

                    # inv = 1/d on DVE (approx, ~51 ULP)
                    inv_t = invp.tile([128, SF], dt.float32, tag="inv")
                    nc.vector.reciprocal_approx_fast(out=inv_t[:, :], in_=d_t[:, :])
                    inv_ap = inv_t[:, :]

                if not fold1:
                    fold_ts.append(out_t)
                    r_t = rp.tile([128, SF], dt.float32, tag="r")
                    reng = nc.gpsimd if r_eng == "pool" else nc.vector
                    reng.tensor_tensor(r_t[:, :], tp_t[:, :], inv_ap, alu.mult)
                    r_ts.append(r_t)
                    continue

                # fold1: prod = out[:, :HF] * out[:, HF:]  (fp16 2x TT mode)
                fold_t = foldp.tile([128, HF], dt.float16, tag="fold")
                nc.vector.tensor_tensor(
                    fold_t[:, :], out_t[:, :HF], out_t[:, HF:], alu.mult
                )

                if fold2:
                    f2_t = fold2p.tile([128, QF], dt.float32, tag="fold2")
                    f2eng = nc.gpsimd if fold2_eng == "pool" else nc.vector
                    f2eng.tensor_tensor(
                        f2_t[:, :], fold_t[:, :QF], fold_t[:, QF:], alu.mult)
                    fold_ts.append(f2_t)
                else:
                    fold_ts.append(fold_t)

                # r = tp * inv
                r_t = rp.tile([128, SF], dt.float32, tag="r")
                reng = nc.gpsimd if r_eng == "pool" else nc.vector
                reng.tensor_tensor(
                    r_t[:, :], tp_t[:, :], inv_ap, alu.mult
                )
                r_ts.append(r_t)

            # ACT phase: batch all Ln then all Exp
            for g in range(G if not dma_only else 0):
                nc.scalar.activation(
                    ldump[:, :], fold_ts[g][:, :], act.Ln,
                    accum_out=logsums_sb[:, g:g + 1],
                )
            for g in range(G if not dma_only else 0):
                nc.scalar.activation(
                    qdump[:, :], r_ts[g][:, :], act.Exp,
                    scale=2.0 / TAU,
                    accum_out=naccs_sb[:, g:g + 1],
                )

        if not dma_only:
            nc.sync.dma_start(norms[:, :], naccs_sb[:, :])
            nc.sync.dma_start(logsums[:, :], logsums_sb[:, :])

    nc.finalize()
    return nc


def _make_consts():
    k = (np.arange(1, SF + 1, dtype=np.float32) * F)  # 16, 32, ..., 4096
    kt = np.ascontiguousarray(np.broadcast_to(k, (128, SF))).astype(np.float32)
    return kt


def _prep_inputs(output, labels):
    """Host-side shard + dtype/layout prep. Returns per-core in_maps."""
    output = np.asarray(output)
    labels = np.asarray(labels)
    assert output.shape == (B, S, 1) and labels.shape == (B, S)

    out_np_dt = ml_dtypes.float8_e4m3 if USE_FP8 else np.float16
    outh_full = (output.reshape(B, S).astype(np.float32, copy=False) * OSCALE
                 ).astype(out_np_dt)
    # fold labels Fx: integer counts 0..F, exact in fp16
    lab8_full = labels.reshape(B, SF, F).sum(axis=2, dtype=np.float32
                                             ).astype(np.float16)

    kt = _make_consts()
    # host inv table: 1/(T_row + k) per folded position, fp16
    T = labels.sum(axis=1, dtype=np.float64)[:, None]          # [B,1]
    kvec = (np.arange(1, SF + 1, dtype=np.float64) * F)[None, :]
    inv_full = (1.0 / (T + kvec)).astype(np.float16)            # [B, SF]
    in_maps = []
    for c in range(NCORES):
        sl = slice(c * RPC, (c + 1) * RPC)
        # outh row-major [RPC, S] (group g = row block, contiguous 1MB DMA);
        # lab8 [128 partitions, G*SF]: col-block g = rows g*128..g*128+127
        lab8_c = np.ascontiguousarray(
            lab8_full[sl].reshape(G, 128, SF).transpose(1, 0, 2).reshape(128, G * SF))
        inv_c = np.ascontiguousarray(
            inv_full[sl].reshape(G, 128, SF).transpose(1, 0, 2).reshape(128, G * SF))
        in_maps.append({
            "outh": np.ascontiguousarray(outh_full[sl]),
            "lab8": lab8_c,
            "invt": inv_c,
            "kt": kt,
        })
    return in_maps


def _postprocess(res):
    total = 0.0
    for c in range(NCORES):
        naccs = np.asarray(res.results[c]["norms"], dtype=np.float64)
        logs = np.asarray(res.results[c]["logsums"], dtype=np.float64)
        total += float(np.sum((logs - LNCORR) / (F * naccs)))
    return np.float32(-total / B)


def _run(output, labels, trace=False):
    from concourse.bass_utils import run_bass_kernel_spmd

    if "prog" not in _PROGRAM_CACHE:
        _PROGRAM_CACHE["prog"] = _build_program()
    nc = _PROGRAM_CACHE["prog"]

    in_maps = _prep_inputs(output, labels)
    res = run_bass_kernel_spmd(nc, in_maps, core_ids=list(range(NCORES)),
                               trace=trace)
    return _postprocess(res), res


def kernel(output, labels):
    loss, _ = _run(output, labels, trace=False)
    return loss


# revision 36
# speedup vs baseline: 1.2715x; 1.0962x over previous
"""AttnCutLoss Trainium2 kernel (v3).

Reference math (B=4096 rows, S=4096 positions, f1 metric, tau=0.95):
    tp    = cumsum(labels, axis=1)
    r     = 2*tp / (k + total)          [exact algebraic form of the f1 weight,
                                         incl. the tp==0 / total==0 guards]
    q     = exp(r/tau); norm = sum_j q; w = 1/norm
    loss  = -sum(log(output)*w)/B = -(1/B) * sum_rows [ (sum_j log(output)) / norm ]

Approximations (tolerance is rel 2e-2; these land ~1e-3 combined):
  * labels are pre-folded Fx on host (F=16): lab16[i] = sum of a 16-block.
    cumsum gives tp at k=16,32,...,4096 EXACTLY; norm ~= 16*sum_i f(16i).
    f = exp(2.105*tp/(T+k)) spans only [1, ~1.43] so the right-endpoint
    Riemann bias is ~+6e-4 relative. 16x fewer scan/recip/mult/exp elems.
  * output is sent as fp16 scaled by 32 (no fp16 subnormals after the first
    pairwise product; ln rel err ~2^-12).
  * log-sum pass is quartered by two pairwise-product folds:
    sum ln(x) = sum ln((a*b)*(c*d)); fold1 fp16 (DVE 2x TT mode),
    fold2 f32 (products up to 32^4 overflow fp16). Host subtracts the exact
    S*ln(32) scale correction per row.

Per-core engine split (512 rows/core, 4 groups of 128 partitions):
  DVE : scan; reciprocal_approx_fast; r=tp*inv; fold1; fold2 (or half)
  ACT : d = k+T via Identity(bias=T); Ln(fold2) accum; Exp(r*2/tau) accum
        (single act-table set 6 serves Ln+Exp: no in-loop table reloads)
  POOL: optional half of fold2 (plain TensorTensor only; TensorScalarPtr on
        Pool costs ~6.5us/op on HW - never use it there)
  DMA : output fp16 4MB/core + labels 0.25MB/core, optionally split across
        the SP and ACT HWDGE rings
Host: loss = -(sum over rows (logsum_row - S*ln 32)/(F*normacc_row))/B.
"""

import numpy as np
import ml_dtypes

B = 4096
S = 4096
TAU = 0.95
NCORES = 8
RPC = B // NCORES          # rows per core = 512
G = RPC // 128             # row groups per core = 4
F = 16                     # host fold factor for labels
SF = S // F                # folded row length = 256
OSCALE = 64.0              # host scale on output before fp8/fp16 cast
USE_FP8 = True             # send output as fp8 e4m3 (else fp16)
LNCORR = S * float(np.log(OSCALE))  # per-row logsum correction

_PROGRAM_CACHE = {}


def _build_program(repeats: int = 1, d_eng: str = "act", r_eng: str = "dve",
                   dma_only: bool = False, static_dma: bool = False,
                   fold2: bool = False, fold2_eng: str = "dve",
                   split_rings: bool = True, dma_pack: int = 1,
                   fp8: bool = USE_FP8, host_inv: bool = False,
                   outp_bufs: int = 3, dma_first: bool = False,
                   fold1: bool = True, unroll: int = 1):
    import concourse.bass as bass
    import concourse.tile as tile
    import concourse.mybir as mybir
    from concourse import bacc
    from contextlib import ExitStack
    import contextlib

    dt = mybir.dt
    alu = mybir.AluOpType
    act = mybir.ActivationFunctionType

    nc = bacc.Bacc("TRN2")
    out_dt = dt.float8e4 if fp8 else dt.float16
    # row-major [RPC, S]: group g = rows [g*128,(g+1)*128) -> contiguous block
    outh = nc.dram_tensor("outh", [RPC, S], out_dt, kind="ExternalInput")
    lab8 = nc.dram_tensor("lab8", [128, G * SF], dt.float16, kind="ExternalInput")
    invt = nc.dram_tensor("invt", [128, G * SF], dt.float16, kind="ExternalInput")
    kt = nc.dram_tensor("kt", [128, SF], dt.float32, kind="ExternalInput")
    norms = nc.dram_tensor("norms", [128, G], dt.float32, kind="ExternalOutput")
    logsums = nc.dram_tensor("logsums", [128, G], dt.float32, kind="ExternalOutput")

    HF = S // 2            # fold1 width
    QF = S // 4            # fold2 width

    with ExitStack() as ctx:
        tc = ctx.enter_context(tile.TileContext(nc))
        consts = ctx.enter_context(tc.tile_pool(name="consts", bufs=1))
        labp = ctx.enter_context(tc.tile_pool(name="labp", bufs=1))
        outp = ctx.enter_context(tc.tile_pool(
            name="outp", bufs=4 if static_dma else outp_bufs))
        tpp = ctx.enter_context(tc.tile_pool(name="tpp", bufs=2))
        dp = ctx.enter_context(tc.tile_pool(name="dp", bufs=2))
        invp = ctx.enter_context(tc.tile_pool(name="invp", bufs=2))
        rp = ctx.enter_context(tc.tile_pool(name="rp", bufs=4))
        foldp = ctx.enter_context(tc.tile_pool(name="foldp", bufs=2))
        fold2p = ctx.enter_context(tc.tile_pool(name="fold2p", bufs=4))
        dump = ctx.enter_context(tc.tile_pool(name="dump", bufs=1))
        accp = ctx.enter_context(tc.tile_pool(name="accp", bufs=1))

        # Pre-load ACT table set 6 (natural_log_exp_and_others): serves BOTH
        # Ln and Exp, so the act-table-load pass inserts no in-loop reloads.
        _li = mybir.InstLoadActFuncSet(
            name=nc.get_next_instruction_name(), ins=[], outs=[])
        _li.act_func_set_id = 6
        nc.scalar.add_instruction(_li)

        kt_sb = consts.tile([128, SF], dt.float32)
        nc.sync.dma_start(kt_sb[:, :], kt[:, :])

        naccs_sb = accp.tile([128, G], dt.float32)
        logsums_sb = accp.tile([128, G], dt.float32)
        qdump = dump.tile([128, SF], dt.bfloat16)
        lnw = (QF if fold2 else HF) if fold1 else S
        ldump = dump.tile([128, lnw], dt.bfloat16)

        def out_dma(g, tile_t):
            eng = nc.scalar if (split_rings and g % 2 == 1) else nc.sync
            eng.dma_start(tile_t[:, :], outh[g * 128:(g + 1) * 128, :])

        static_outs = []
        if static_dma:
            lab_t = labp.tile([128, G * SF], dt.float16, tag="lab")
            nc.sync.dma_start(lab_t[:, :], lab8[:, :])
            for g in range(G):
                sout = outp.tile([128, S], out_dt, tag="outv")
                out_dma(g, sout)
                static_outs.append(sout)

        loop_cm = tc.For_i(0, repeats // unroll, 1) if repeats > 1             else contextlib.nullcontext()
        with loop_cm:
          for _u in range(unroll):
            if not static_dma:
                lab_t = labp.tile([128, G * SF], dt.float16, tag="lab")
                nc.sync.dma_start(lab_t[:, :], lab8[:, :])
            if host_inv:
                inv_all = labp.tile([128, G * SF], dt.float16, tag="invh")
                nc.sync.dma_start(inv_all[:, :], invt[:, :])
            if dma_only:
                assert dma_pack in (1, 2, 4)
                npk = G // dma_pack
                for i in range(npk):
                    out_t = outp.tile([128, S * dma_pack], out_dt, tag="outv")
                    eng = nc.scalar if (split_rings and i % 2 == 1) else nc.sync
                    src = outh[i * 128 * dma_pack:(i + 1) * 128 * dma_pack, :]
                    if dma_pack > 1:
                        src = src.rearrange("(k p) s -> p k s", k=dma_pack)
                    eng.dma_start(out_t[:, :], src)
            fold_ts = []
            r_ts = []
            pre_outs = []
            if dma_first and not static_dma and not dma_only:
                for g in range(G):
                    out_t = outp.tile([128, S], out_dt, tag="outv")
                    out_dma(g, out_t)
                    pre_outs.append(out_t)
            if dma_pack > 1 and not static_dma and not dma_only:
                # packed DMAs: tile cols [j*S:(j+1)*S] = group i*pack+j
                for i in range(G // dma_pack):
                    pt = outp.tile([128, S * dma_pack], out_dt, tag="outv")
                    eng = nc.scalar if (split_rings and i % 2 == 1) else nc.sync
                    src = outh[i * 128 * dma_pack:(i + 1) * 128 * dma_pack, :]
                    src = src.rearrange("(k p) s -> p k s", k=dma_pack)
                    eng.dma_start(pt[:, :], src)
                    pre_outs.append(pt)
            for g in range(G if not dma_only else 0):
                if static_dma:
                    out_t = static_outs[g]
                elif dma_first:
                    out_t = pre_outs[g]
                elif dma_pack > 1:
                    out_t = pre_outs[g // dma_pack][
                        :, (g % dma_pack) * S:(g % dma_pack + 1) * S]
                else:
                    out_t = outp.tile([128, S], out_dt, tag="outv")
                    out_dma(g, out_t)

                # tp = cumsum(lab) along free dim; exact integers
                tp_t = tpp.tile([128, SF], dt.float32, tag="tp")
                nc.vector.tensor_tensor_scan(
                    tp_t[:, :], lab_t[:, g * SF:(g + 1) * SF],
                    lab_t[:, g * SF:(g + 1) * SF], 0.0, alu.add, alu.bypass
                )

                if host_inv:
                    inv_ap = inv_all[:, g * SF:(g + 1) * SF]
                else:
                    # d = k + T  (T = tp[:, -1], per-partition scalar)
                    d_t = dp.tile([128, SF], dt.float32, tag="d")
                    if d_eng == "act":
                        nc.scalar.activation(
                            d_t[:, :], kt_sb[:, :], act.Identity,
                            bias=tp_t[:, SF - 1:SF], scale=1.0)
                    else:
                        deng = nc.gpsimd if d_eng == "pool" else nc.vector
                        deng.tensor_scalar_add(d_t[:, :], kt_sb[:, :],
                                               tp_t[:, SF - 1:SF])

                    # inv = 1/d on DVE (approx, ~51 ULP)
                    inv_t = invp.tile([128, SF], dt.float32, tag="inv")
                    nc.vector.reciprocal_approx_fast(out=inv_t[:, :], in_=d_t[:, :])
                    inv_ap = inv_t[:, :]

                if not fold1:
                    fold_ts.append(out_t)
                    r_t = rp.tile([128, SF], dt.float32, tag="r")
                    reng = nc.gpsimd if r_eng == "pool" else nc.vector
                    reng.tensor_tensor(r_t[:, :], tp_t[:, :], inv_ap, alu.mult)
                    r_ts.append(r_t)
                    continue

                # fold1: prod = out[:, :HF] * out[:, HF:]  (fp16 2x TT mode)
                fold_t = foldp.tile([128, HF], dt.float16, tag="fold")
                nc.vector.tensor_tensor(
                    fold_t[:, :], out_t[:, :HF], out_t[:, HF:], alu.mult
                )

                if fold2:
                    f2_t = fold2p.tile([128, QF], dt.float32, tag="fold2")
                    f2eng = nc.gpsimd if fold2_eng == "pool" else nc.vector
                    f2eng.tensor_tensor(
                        f2_t[:, :], fold_t[:, :QF], fold_t[:, QF:], alu.mult)
                    fold_ts.append(f2_t)
                else:
                    fold_ts.append(fold_t)

                # r = tp * inv
                r_t = rp.tile([128, SF], dt.float32, tag="r")
                reng = nc.gpsimd if r_eng == "pool" else nc.vector
                reng.tensor_tensor(
                    r_t[:, :], tp_t[:, :], inv_ap, alu.mult
                )
                r_ts.append(r_t)

            # ACT phase: batch all Ln then all Exp
            for g in range(G if not dma_only else 0):
                nc.scalar.activation(
                    ldump[:, :], fold_ts[g][:, :], act.Ln,
                    accum_out=logsums_sb[:, g:g + 1],
                )
            for g in range(G if not dma_only else 0):
                nc.scalar.activation(
                    qdump[:, :], r_ts[g][:, :], act.Exp,
                    scale=2.0 / TAU,
                    accum_out=naccs_sb[:, g:g + 1],
                )

        if not dma_only:
            nc.sync.dma_start(norms[:, :], naccs_sb[:, :])
            nc.sync.dma_start(logsums[:, :], logsums_sb[:, :])

    nc.finalize()
    return nc


def _make_consts():
    k = (np.arange(1, SF + 1, dtype=np.float32) * F)  # 16, 32, ..., 4096
    kt = np.ascontiguousarray(np.broadcast_to(k, (128, SF))).astype(np.float32)
    return kt


def _prep_inputs(output, labels):
    """Host-side shard + dtype/layout prep. Returns per-core in_maps."""
    output = np.asarray(output)
    labels = np.asarray(labels)
    assert output.shape == (B, S, 1) and labels.shape == (B, S)

    out_np_dt = ml_dtypes.float8_e4m3 if USE_FP8 else np.float16
    outh_full = (output.reshape(B, S).astype(np.float32, copy=False) * OSCALE
                 ).astype(out_np_dt)
    # fold labels Fx: integer counts 0..F, exact in fp16
    lab8_full = labels.reshape(B, SF, F).sum(axis=2, dtype=np.float32
                                             ).astype(np.float16)

    kt = _make_consts()
    # host inv table: 1/(T_row + k) per folded position, fp16
    T = labels.sum(axis=1, dtype=np.float64)[:, None]          # [B,1]
    kvec = (np.arange(1, SF + 1, dtype=np.float64) * F)[None, :]
    inv_full = (1.0 / (T + kvec)).astype(np.float16)            # [B, SF]
    in_maps = []
    for c in range(NCORES):
        sl = slice(c * RPC, (c + 1) * RPC)
        # outh row-major [RPC, S] (group g = row block, contiguous 1MB DMA);
        # lab8 [128 partitions, G*SF]: col-block g = rows g*128..g*128+127
        lab8_c = np.ascontiguousarray(
            lab8_full[sl].reshape(G, 128, SF).transpose(1, 0, 2).reshape(128, G * SF))
        inv_c = np.ascontiguousarray(
            inv_full[sl].reshape(G, 128, SF).transpose(1, 0, 2).reshape(128, G * SF))
        in_maps.append({
            "outh": np.ascontiguousarray(outh_full[sl]),
            "lab8": lab8_c,
            "invt": inv_c,
            "kt": kt,
        })
    return in_maps


def _postprocess(res):
    total = 0.0
    for c in range(NCORES):
        naccs = np.asarray(res.results[c]["norms"], dtype=np.float64)
        logs = np.asarray(res.results[c]["logsums"], dtype=np.float64)
        total += float(np.sum((logs - LNCORR) / (F * naccs)))
    return np.float32(-total / B)


def _run(output, labels, trace=False):
    from concourse.bass_utils import run_bass_kernel_spmd

    if "prog" not in _PROGRAM_CACHE:
        _PROGRAM_CACHE["prog"] = _build_program()
    nc = _PROGRAM_CACHE["prog"]

    in_maps = _prep_inputs(output, labels)
    res = run_bass_kernel_spmd(nc, in_maps, core_ids=list(range(NCORES)),
                               trace=trace)
    return _postprocess(res), res


def kernel(output, labels):
    loss, _ = _run(output, labels, trace=False)
    return loss


# revision 38
# speedup vs baseline: 1.3047x; 1.0262x over previous
"""AttnCutLoss Trainium2 kernel (v3).

Reference math (B=4096 rows, S=4096 positions, f1 metric, tau=0.95):
    tp    = cumsum(labels, axis=1)
    r     = 2*tp / (k + total)          [exact algebraic form of the f1 weight,
                                         incl. the tp==0 / total==0 guards]
    q     = exp(r/tau); norm = sum_j q; w = 1/norm
    loss  = -sum(log(output)*w)/B = -(1/B) * sum_rows [ (sum_j log(output)) / norm ]

Approximations (tolerance is rel 2e-2; these land ~1e-3 combined):
  * labels are pre-folded Fx on host (F=16): lab16[i] = sum of a 16-block.
    cumsum gives tp at k=16,32,...,4096 EXACTLY; norm ~= 16*sum_i f(16i).
    f = exp(2.105*tp/(T+k)) spans only [1, ~1.43] so the right-endpoint
    Riemann bias is ~+6e-4 relative. 16x fewer scan/recip/mult/exp elems.
  * output is sent as fp16 scaled by 32 (no fp16 subnormals after the first
    pairwise product; ln rel err ~2^-12).
  * log-sum pass is quartered by two pairwise-product folds:
    sum ln(x) = sum ln((a*b)*(c*d)); fold1 fp16 (DVE 2x TT mode),
    fold2 f32 (products up to 32^4 overflow fp16). Host subtracts the exact
    S*ln(32) scale correction per row.

Per-core engine split (512 rows/core, 4 groups of 128 partitions):
  DVE : scan; reciprocal_approx_fast; r=tp*inv; fold1; fold2 (or half)
  ACT : d = k+T via Identity(bias=T); Ln(fold2) accum; Exp(r*2/tau) accum
        (single act-table set 6 serves Ln+Exp: no in-loop table reloads)
  POOL: optional half of fold2 (plain TensorTensor only; TensorScalarPtr on
        Pool costs ~6.5us/op on HW - never use it there)
  DMA : output fp16 4MB/core + labels 0.25MB/core, optionally split across
        the SP and ACT HWDGE rings
Host: loss = -(sum over rows (logsum_row - S*ln 32)/(F*normacc_row))/B.
"""

import numpy as np
import ml_dtypes

B = 4096
S = 4096
TAU = 0.95
NCORES = 8
RPC = B // NCORES          # rows per core = 512
G = RPC // 128             # row groups per core = 4
F = 16                     # host fold factor for labels
SF = S // F                # folded row length = 256
OSCALE = 64.0              # host scale on output before fp8/fp16 cast
USE_FP8 = True             # send output as fp8 e4m3 (else fp16)
LNCORR = S * float(np.log(OSCALE))  # per-row logsum correction

_PROGRAM_CACHE = {}


def _build_program(repeats: int = 1, d_eng: str = "act", r_eng: str = "dve",
                   dma_only: bool = False, static_dma: bool = False,
                   fold2: bool = False, fold2_eng: str = "dve",
                   split_rings: bool = True, dma_pack: int = 1,
                   fp8: bool = USE_FP8, host_inv: bool = False,
                   outp_bufs: int = 3, dma_first: bool = False,
                   fold1: bool = True, unroll: int = 1,
                   tail_split: bool = True):
    import concourse.bass as bass
    import concourse.tile as tile
    import concourse.mybir as mybir
    from concourse import bacc
    from contextlib import ExitStack
    import contextlib

    dt = mybir.dt
    alu = mybir.AluOpType
    act = mybir.ActivationFunctionType

    nc = bacc.Bacc("TRN2")
    out_dt = dt.float8e4 if fp8 else dt.float16
    # row-major [RPC, S]: group g = rows [g*128,(g+1)*128) -> contiguous block
    outh = nc.dram_tensor("outh", [RPC, S], out_dt, kind="ExternalInput")
    lab8 = nc.dram_tensor("lab8", [128, G * SF], dt.float16, kind="ExternalInput")
    invt = nc.dram_tensor("invt", [128, G * SF], dt.float16, kind="ExternalInput")
    kt = nc.dram_tensor("kt", [128, SF], dt.float32, kind="ExternalInput")
    norms = nc.dram_tensor("norms", [128, G], dt.float32, kind="ExternalOutput")
    NLS = G + 1 if tail_split else G
    logsums = nc.dram_tensor("logsums", [128, NLS], dt.float32, kind="ExternalOutput")

    HF = S // 2            # fold1 width
    QF = S // 4            # fold2 width

    with ExitStack() as ctx:
        tc = ctx.enter_context(tile.TileContext(nc))
        consts = ctx.enter_context(tc.tile_pool(name="consts", bufs=1))
        labp = ctx.enter_context(tc.tile_pool(name="labp", bufs=1))
        outp = ctx.enter_context(tc.tile_pool(
            name="outp", bufs=4 if static_dma else outp_bufs))
        tpp = ctx.enter_context(tc.tile_pool(name="tpp", bufs=2))
        dp = ctx.enter_context(tc.tile_pool(name="dp", bufs=2))
        invp = ctx.enter_context(tc.tile_pool(name="invp", bufs=2))
        rp = ctx.enter_context(tc.tile_pool(name="rp", bufs=4))
        foldp = ctx.enter_context(tc.tile_pool(name="foldp", bufs=2))
        fold2p = ctx.enter_context(tc.tile_pool(name="fold2p", bufs=4))
        dump = ctx.enter_context(tc.tile_pool(name="dump", bufs=1))
        accp = ctx.enter_context(tc.tile_pool(name="accp", bufs=1))

        # Pre-load ACT table set 6 (natural_log_exp_and_others): serves BOTH
        # Ln and Exp, so the act-table-load pass inserts no in-loop reloads.
        _li = mybir.InstLoadActFuncSet(
            name=nc.get_next_instruction_name(), ins=[], outs=[])
        _li.act_func_set_id = 6
        nc.scalar.add_instruction(_li)

        kt_sb = consts.tile([128, SF], dt.float32)
        nc.sync.dma_start(kt_sb[:, :], kt[:, :])

        naccs_sb = accp.tile([128, G], dt.float32)
        logsums_sb = accp.tile([128, NLS], dt.float32)
        qdump = dump.tile([128, SF], dt.bfloat16)
        lnw = (QF if fold2 else HF) if fold1 else S
        ldump = dump.tile([128, lnw], dt.bfloat16)

        def out_dma(g, tile_t):
            eng = nc.scalar if (split_rings and g % 2 == 1) else nc.sync
            eng.dma_start(tile_t[:, :], outh[g * 128:(g + 1) * 128, :])

        static_outs = []
        if static_dma:
            lab_t = labp.tile([128, G * SF], dt.float16, tag="lab")
            nc.sync.dma_start(lab_t[:, :], lab8[:, :])
            for g in range(G):
                sout = outp.tile([128, S], out_dt, tag="outv")
                out_dma(g, sout)
                static_outs.append(sout)

        loop_cm = tc.For_i(0, repeats // unroll, 1) if repeats > 1             else contextlib.nullcontext()
        with loop_cm:
          for _u in range(unroll):
            if not static_dma:
                lab_t = labp.tile([128, G * SF], dt.float16, tag="lab")
                nc.sync.dma_start(lab_t[:, :], lab8[:, :])
            if host_inv:
                inv_all = labp.tile([128, G * SF], dt.float16, tag="invh")
                nc.sync.dma_start(inv_all[:, :], invt[:, :])
            if dma_only:
                assert dma_pack in (1, 2, 4)
                npk = G // dma_pack
                for i in range(npk):
                    out_t = outp.tile([128, S * dma_pack], out_dt, tag="outv")
                    eng = nc.scalar if (split_rings and i % 2 == 1) else nc.sync
                    src = outh[i * 128 * dma_pack:(i + 1) * 128 * dma_pack, :]
                    if dma_pack > 1:
                        src = src.rearrange("(k p) s -> p k s", k=dma_pack)
                    eng.dma_start(out_t[:, :], src)
            fold_ts = []
            r_ts = []
            pre_outs = []
            if dma_first and not static_dma and not dma_only:
                for g in range(G):
                    out_t = outp.tile([128, S], out_dt, tag="outv")
                    out_dma(g, out_t)
                    pre_outs.append(out_t)
            if dma_pack > 1 and not static_dma and not dma_only:
                # packed DMAs: tile cols [j*S:(j+1)*S] = group i*pack+j
                for i in range(G // dma_pack):
                    pt = outp.tile([128, S * dma_pack], out_dt, tag="outv")
                    eng = nc.scalar if (split_rings and i % 2 == 1) else nc.sync
                    src = outh[i * 128 * dma_pack:(i + 1) * 128 * dma_pack, :]
                    src = src.rearrange("(k p) s -> p k s", k=dma_pack)
                    eng.dma_start(pt[:, :], src)
                    pre_outs.append(pt)
            for g in range(G if not dma_only else 0):
                if static_dma:
                    out_t = static_outs[g]
                elif dma_first:
                    out_t = pre_outs[g]
                elif dma_pack > 1:
                    out_t = pre_outs[g // dma_pack][
                        :, (g % dma_pack) * S:(g % dma_pack + 1) * S]
                elif tail_split and g == G - 1:
                    out_t = None
                    oh0 = outp.tile([128, HF], out_dt, tag="outh0")
                    oh1 = outp.tile([128, HF], out_dt, tag="outh1")
                    nc.sync.dma_start(oh0[:, :], outh[g * 128:(g + 1) * 128, :HF])
                    eng2 = nc.scalar if split_rings else nc.sync
                    eng2.dma_start(oh1[:, :], outh[g * 128:(g + 1) * 128, HF:])
                else:
                    out_t = outp.tile([128, S], out_dt, tag="outv")
                    out_dma(g, out_t)

                # tp = cumsum(lab) along free dim; exact integers
                tp_t = tpp.tile([128, SF], dt.float32, tag="tp")
                nc.vector.tensor_tensor_scan(
                    tp_t[:, :], lab_t[:, g * SF:(g + 1) * SF],
                    lab_t[:, g * SF:(g + 1) * SF], 0.0, alu.add, alu.bypass
                )

                if host_inv:
                    inv_ap = inv_all[:, g * SF:(g + 1) * SF]
                else:
                    # d = k + T  (T = tp[:, -1], per-partition scalar)
                    d_t = dp.tile([128, SF], dt.float32, tag="d")
                    if d_eng == "act":
                        nc.scalar.activation(
                            d_t[:, :], kt_sb[:, :], act.Identity,
                            bias=tp_t[:, SF - 1:SF], scale=1.0)
                    else:
                        deng = nc.gpsimd if d_eng == "pool" else nc.vector
                        deng.tensor_scalar_add(d_t[:, :], kt_sb[:, :],
                                               tp_t[:, SF - 1:SF])

                    # inv = 1/d on DVE (approx, ~51 ULP)
                    inv_t = invp.tile([128, SF], dt.float32, tag="inv")
                    nc.vector.reciprocal_approx_fast(out=inv_t[:, :], in_=d_t[:, :])
                    inv_ap = inv_t[:, :]

                if not fold1:
                    fold_ts.append(out_t)
                    r_t = rp.tile([128, SF], dt.float32, tag="r")
                    reng = nc.gpsimd if r_eng == "pool" else nc.vector
                    reng.tensor_tensor(r_t[:, :], tp_t[:, :], inv_ap, alu.mult)
                    r_ts.append(r_t)
                    continue

                # fold1: prod = out[:, :HF] * out[:, HF:]  (fp16 2x TT mode)
                fold_t = foldp.tile([128, HF], dt.float16, tag="fold")
                if tail_split and g == G - 1:
                    # intra-half pairing: each half folds independently so
                    # fold/Ln start as soon as its own half-DMA lands
                    nc.vector.tensor_tensor(
                        fold_t[:, :QF], oh0[:, :QF], oh0[:, QF:], alu.mult)
                    nc.vector.tensor_tensor(
                        fold_t[:, QF:], oh1[:, :QF], oh1[:, QF:], alu.mult)
                else:
                    nc.vector.tensor_tensor(
                        fold_t[:, :], out_t[:, :HF], out_t[:, HF:], alu.mult
                    )

                if fold2:
                    f2_t = fold2p.tile([128, QF], dt.float32, tag="fold2")
                    f2eng = nc.gpsimd if fold2_eng == "pool" else nc.vector
                    f2eng.tensor_tensor(
                        f2_t[:, :], fold_t[:, :QF], fold_t[:, QF:], alu.mult)
                    fold_ts.append(f2_t)
                else:
                    fold_ts.append(fold_t)

                # r = tp * inv
                r_t = rp.tile([128, SF], dt.float32, tag="r")
                reng = nc.gpsimd if r_eng == "pool" else nc.vector
                reng.tensor_tensor(
                    r_t[:, :], tp_t[:, :], inv_ap, alu.mult
                )
                r_ts.append(r_t)

            # ACT phase: batch all Ln then all Exp
            for g in range(G if not dma_only else 0):
                if tail_split and g == G - 1:
                    nc.scalar.activation(
                        ldump[:, :QF], fold_ts[g][:, :QF], act.Ln,
                        accum_out=logsums_sb[:, g:g + 1],
                    )
                    nc.scalar.activation(
                        ldump[:, QF:HF], fold_ts[g][:, QF:], act.Ln,
                        accum_out=logsums_sb[:, G:G + 1],
                    )
                else:
                    nc.scalar.activation(
                        ldump[:, :], fold_ts[g][:, :], act.Ln,
                        accum_out=logsums_sb[:, g:g + 1],
                    )
            for g in range(G if not dma_only else 0):
                nc.scalar.activation(
                    qdump[:, :], r_ts[g][:, :], act.Exp,
                    scale=2.0 / TAU,
                    accum_out=naccs_sb[:, g:g + 1],
                )

        if not dma_only:
            nc.sync.dma_start(norms[:, :], naccs_sb[:, :])
            nc.sync.dma_start(logsums[:, :], logsums_sb[:, :])

    nc.finalize()
    return nc


def _make_consts():
    k = (np.arange(1, SF + 1, dtype=np.float32) * F)  # 16, 32, ..., 4096
    kt = np.ascontiguousarray(np.broadcast_to(k, (128, SF))).astype(np.float32)
    return kt


def _prep_inputs(output, labels):
    """Host-side shard + dtype/layout prep. Returns per-core in_maps."""
    output = np.asarray(output)
    labels = np.asarray(labels)
    assert output.shape == (B, S, 1) and labels.shape == (B, S)

    out_np_dt = ml_dtypes.float8_e4m3 if USE_FP8 else np.float16
    outh_full = (output.reshape(B, S).astype(np.float32, copy=False) * OSCALE
                 ).astype(out_np_dt)
    # fold labels Fx: integer counts 0..F, exact in fp16
    lab8_full = labels.reshape(B, SF, F).sum(axis=2, dtype=np.float32
                                             ).astype(np.float16)

    kt = _make_consts()
    # host inv table: 1/(T_row + k) per folded position, fp16
    T = labels.sum(axis=1, dtype=np.float64)[:, None]          # [B,1]
    kvec = (np.arange(1, SF + 1, dtype=np.float64) * F)[None, :]
    inv_full = (1.0 / (T + kvec)).astype(np.float16)            # [B, SF]
    in_maps = []
    for c in range(NCORES):
        sl = slice(c * RPC, (c + 1) * RPC)
        # outh row-major [RPC, S] (group g = row block, contiguous 1MB DMA);
        # lab8 [128 partitions, G*SF]: col-block g = rows g*128..g*128+127
        lab8_c = np.ascontiguousarray(
            lab8_full[sl].reshape(G, 128, SF).transpose(1, 0, 2).reshape(128, G * SF))
        inv_c = np.ascontiguousarray(
            inv_full[sl].reshape(G, 128, SF).transpose(1, 0, 2).reshape(128, G * SF))
        in_maps.append({
            "outh": np.ascontiguousarray(outh_full[sl]),
            "lab8": lab8_c,
            "invt": inv_c,
            "kt": kt,
        })
    return in_maps


def _postprocess(res):
    total = 0.0
    for c in range(NCORES):
        naccs = np.asarray(res.results[c]["norms"], dtype=np.float64)
        logs = np.asarray(res.results[c]["logsums"], dtype=np.float64)
        if logs.shape[1] > G:
            logs = np.concatenate(
                [logs[:, :G - 1], (logs[:, G - 1] + logs[:, G])[:, None]], axis=1)
        total += float(np.sum((logs - LNCORR) / (F * naccs)))
    return np.float32(-total / B)


def _run(output, labels, trace=False):
    from concourse.bass_utils import run_bass_kernel_spmd

    if "prog" not in _PROGRAM_CACHE:
        _PROGRAM_CACHE["prog"] = _build_program()
    nc = _PROGRAM_CACHE["prog"]

    in_maps = _prep_inputs(output, labels)
    res = run_bass_kernel_spmd(nc, in_maps, core_ids=list(range(NCORES)),
                               trace=trace)
    return _postprocess(res), res


def kernel(output, labels):
    loss, _ = _run(output, labels, trace=False)
    return loss
